# revision 8
# baseline (speedup 1.0000x reference)
"""Trainium2 Bass kernel for nn_ArgumentClassification (2-layer BiLSTM tagger).

v2: direction-sharded. 8 cores = 4 batch groups x 2 directions. Each core
runs ONE LSTM direction for its 8-row batch group, so the sequential scans
stream half the recurrent weights per core vs the f/b-interleaved baseline
(the scan is LDWEIGHTS-issue bound at ~34ns per [128x128] weight tile).

Per-core symmetric program (no control divergence; asymmetry lives in the
host-provided data):
  - odd cores (backward dir) receive time-REVERSED hidden_states/features
    and their direction's weights under the same tensor names, so a single
    "forward scan" program computes both directions.
  - after each layer's scan, h is exchanged within the core pair via an
    HBM AllGather over replica groups [[0,1],[2,3],[4,5],[6,7]]; each core
    imports BOTH slots contiguously and selects the partner slot with
    host-provided 0/1 mask scalars while time-reversing via negative-
    stride DVE reads (3 DVE ops).
  - L1/out-proj weights have their k-blocks host-permuted to [own-dir;
    partner-dir], so the device always builds x1 = [h_own; h_recv].
  - even cores produce the true-time output; odd cores' outputs are
    time-reversed and ignored by the host.

The scan keeps PyTorch's gate layout (i,f,g,o), accumulates i+f, g and
o in separate PSUM banks (groups emitted in that order), folds the xw
input-projection term into each PSUM group with one identity matmul so
activations read PSUM directly, and interleaves the next chunks of the
input projections into the per-step PE tail as fillers.
"""
import sys

sys.path.insert(0, "/opt/trn_rl_repo")

import numpy as np
import ml_dtypes

import concourse.bass as bass
import concourse.tile as tile
from concourse import bacc, mybir
from concourse.bass import ds
from concourse.masks import make_identity

BF16 = mybir.dt.bfloat16
F32 = mybir.dt.float32
AF = mybir.ActivationFunctionType
OP = mybir.AluOpType

B, S, E, H, C = 32, 256, 768, 512, 30
NCORES = 8
NPAIR = NCORES // 2          # 4 batch groups
BL = B // NPAIR              # 8 rows per core
SB = S * BL                  # 2048 columns, ordered (t, b): col = t*BL + b
G = 4 * H                    # 2048 gate rows
MT = G // 128                # 16 gate m-tiles
KH = H // 128                # 4 hidden k-tiles
K0 = 7                       # L0 input k-tiles ([770 + ones-row] -> 896)
K1 = 9                       # L1 input k-tiles (1024 + ones-row -> 1152)
GROUPS = [[2 * g, 2 * g + 1] for g in range(NPAIR)]

_cache = {}


def _bf(a):
    return np.asarray(a, dtype=ml_dtypes.bfloat16)


def _prep_weights(inp):
    """Host-side, per direction d: permute gates to (i,f,o,g), transpose,
    pad, fold biases, tile for SBUF. k-blocks of L1/out weights ordered
    [own-dir, partner-dir]. Returns dict d -> {name -> np array}."""
    perm = np.arange(4 * H)       # keep PyTorch order i,f,g,o

    def tile_k(a, nk):
        # [nk*128, M] -> [128, nk, M]
        return np.ascontiguousarray(
            a.reshape(nk, 128, a.shape[1]).transpose(1, 0, 2))

    def tile_km(a, nk):
        # [nk*128, 16*128] -> [16, 128, nk, 128]  (per-m-block contiguous)
        m = a.shape[1] // 128
        return np.ascontiguousarray(
            a.reshape(nk, 128, m, 128).transpose(2, 1, 0, 3))

    wm = {"f": {}, "b": {}}
    for d, od in (("f", "b"), ("b", "f")):
        o = wm[d]
        # layer 0
        wih = inp[f"Wih_l0{d}"][perm]                     # [2048, 770]
        bias = (inp[f"bih_l0{d}"] + inp[f"bhh_l0{d}"])[perm]
        ext = np.zeros((K0 * 128, G), np.float32)
        ext[:768] = wih.T[:768]
        ext[768] = wih.T[768]      # delta coeffs at tile6 partition 0
        ext[800] = wih.T[769]      # mask coeffs at tile6 partition 32
        ext[832] = bias            # bias row at tile6 partition 64
        o["wih0"] = _bf(tile_km(ext, K0))                 # [16,128,7,128]
        whh = inp[f"Whh_l0{d}"][perm]                     # [2048, 512]
        o["whh0"] = _bf(tile_k(whh.T, KH))                # [128, 4, 2048]
        # layer 1: k-blocks [own 512, partner 512, bias]
        wih = inp[f"Wih_l1{d}"][perm]                     # [2048, 1024]
        own_half = wih.T[:512] if d == "f" else wih.T[512:]
        par_half = wih.T[512:] if d == "f" else wih.T[:512]
        bias = (inp[f"bih_l1{d}"] + inp[f"bhh_l1{d}"])[perm]
        ext = np.zeros((K1 * 128, G), np.float32)
        ext[:512] = own_half
        ext[512:1024] = par_half
        ext[1024] = bias
        o["wih1"] = _bf(tile_km(ext, K1))                 # [16,128,9,128]
        whh = inp[f"Whh_l1{d}"][perm]
        o["whh1"] = _bf(tile_k(whh.T, KH))
        # output projection partial: [own h1 half, bias/2] -> exchange of
        # [30, SB] partials instead of full h1 (8x less traffic)
        wt = inp["W_out"].T                               # [1024, 30]
        ext = np.zeros((5 * 128, C), np.float32)
        ext[:512] = wt[:512] if d == "f" else wt[512:]
        ext[512] = inp["b_out"] * 0.5
        o["wout"] = _bf(tile_k(ext, 5))                   # [128, 5, 30]
    return wm


def build_nc():
    nc = bacc.Bacc("TRN2", target_bir_lowering=False, debug=False,
                   num_devices=NCORES)
    hs = nc.dram_tensor("hs", [4, BL, S, E], BF16, kind="ExternalInput").ap()
    mask = nc.dram_tensor("mask", [1, SB], BF16, kind="ExternalInput").ap()
    oh = nc.dram_tensor("oh", [1, SB], F32, kind="ExternalInput").ap()
    msel = nc.dram_tensor("msel", [128, 2], F32, kind="ExternalInput").ap()
    w = {}
    w["wih0"] = nc.dram_tensor("wih0", [MT, 128, K0, 128], BF16,
                               kind="ExternalInput").ap()
    w["wih1"] = nc.dram_tensor("wih1", [MT, 128, K1, 128], BF16,
                               kind="ExternalInput").ap()
    w["whh0"] = nc.dram_tensor("whh0", [128, KH, G], BF16,
                               kind="ExternalInput").ap()
    w["whh1"] = nc.dram_tensor("whh1", [128, KH, G], BF16,
                               kind="ExternalInput").ap()
    w["wout"] = nc.dram_tensor("wout", [128, 5, C], BF16,
                               kind="ExternalInput").ap()
    out = nc.dram_tensor("out", [BL, S, C], F32, kind="ExternalOutput").ap()

    with tile.TileContext(nc) as tc:
        _emit(nc, tc, hs, mask, oh, msel, w, out)
    nc.compile()
    return nc


def _emit(nc, tc, hs, mask, oh, msel, w, out):
    from contextlib import ExitStack
    with ExitStack() as st:
        cpool = st.enter_context(tc.tile_pool(name="const", bufs=1))
        rpool = st.enter_context(tc.tile_pool(name="rows", bufs=1))
        scpool = st.enter_context(tc.tile_pool(name="sc", bufs=3))
        wpool = st.enter_context(tc.tile_pool(name="wts", bufs=2))
        whpool = st.enter_context(tc.tile_pool(name="whh", bufs=1))
        xwpool = st.enter_context(tc.tile_pool(name="xw", bufs=1))
        pspool = st.enter_context(tc.tile_pool(name="ps", bufs=2, space="PSUM"))
        psg = st.enter_context(tc.tile_pool(name="psg", bufs=1, space="PSUM"))
        dpool = st.enter_context(tc.tile_pool(name="dram", bufs=1,
                                              space="DRAM"))
        fst = ExitStack()
        hlpool = fst.enter_context(tc.tile_pool(name="hl", bufs=8))
        sumpool = fst.enter_context(tc.tile_pool(name="sum", bufs=4))
        fpool = fst.enter_context(tc.tile_pool(name="feat", bufs=1))
        xpool = None

        ident = cpool.tile([128, 128], F32, tag="ident")
        make_identity(nc, ident[:, :])
        identb = cpool.tile([128, 128], BF16, tag="identb")
        nc.vector.tensor_copy(identb[:, :], ident[:, :])
        ones_col = cpool.tile([128, 1], BF16, tag="onescol")
        nc.vector.memset(ones_col[:, :], 1.0)
        ones_row = cpool.tile([128, 512], BF16, tag="onesrow")
        nc.vector.memset(ones_row[:, :], 0.0)
        nc.vector.memset(ones_row[0:1, :], 1.0)
        mscal = cpool.tile([128, 2], F32, tag="mscal")
        nc.sync.dma_start(out=mscal[:, :], in_=msel[:, :])

        # ---- x.T construction: [128, 7, SB] bf16 -------------------------
        xt = fpool.tile([128, K0, SB], BF16, tag="xt")
        hs_sbe = hs.rearrange("l b s e -> l s b e")
        NR = SB // 128            # 16 row-tiles of (t,b)
        SR = 128 // BL            # 16 s-rows per tile
        for r in range(NR):
            hl = []
            for layer in range(4):
                t = hlpool.tile([128, E], BF16, tag="hl")
                nc.sync.dma_start(out=t[:, :],
                                  in_=hs_sbe[layer, SR * r:SR * (r + 1), :, :])
                hl.append(t)
            s01 = sumpool.tile([128, E], F32, tag="sum")
            nc.vector.tensor_tensor(s01[:, :], hl[0][:, :], hl[1][:, :], OP.add)
            s23 = sumpool.tile([128, E], F32, tag="sum")
            nc.vector.tensor_tensor(s23[:, :], hl[2][:, :], hl[3][:, :], OP.add)
            ssum = sumpool.tile([128, E], F32, tag="sum")
            nc.vector.tensor_tensor(ssum[:, :], s01[:, :], s23[:, :], OP.add)
            for c in range(6):
                pt = pspool.tile([128, 128], F32, tag="tp")
                nc.tensor.transpose(pt[:, :], ssum[:, 128 * c:128 * (c + 1)],
                                    ident[:, :])
                nc.vector.tensor_scalar_mul(
                    xt[:, c, 128 * r:128 * (r + 1)], pt[:, :], 0.25)

        # ---- feature rows (delta, mask, ones) in xt[:, 6, :] -------------
        nc.vector.memset(xt[:, 6, :], 0.0)
        nc.vector.memset(xt[64:65, 6, :], 1.0)
        nc.sync.dma_start(out=xt[32:33, 6, :], in_=mask[:, :])

        # mean_word row via ones-matmul over the 6 full e-tiles
        mw = fpool.tile([1, SB], F32, tag="mw")
        for ch in range(4):
            mp_ps = pspool.tile([1, 512], F32, tag="proj")
            for k in range(6):
                nc.tensor.matmul(mp_ps[:, :], ones_col[:, :],
                                 xt[:, k, 512 * ch:512 * (ch + 1)],
                                 start=(k == 0), stop=(k == 5))
            nc.vector.tensor_scalar_mul(mw[0:1, 512 * ch:512 * (ch + 1)],
                                        mp_ps[:, :], 1.0 / E)
        # mean at first-predicate position: host sends the one-hot
        ohrow = fpool.tile([1, SB], F32, tag="ohrow")
        nc.sync.dma_start(out=ohrow[:, :], in_=oh[:, :])
        nc.vector.tensor_tensor(ohrow[:, :], ohrow[:, :], mw[:, :], OP.mult)
        mpred = fpool.tile([1, BL], F32, tag="mpred")
        oh_b = ohrow.rearrange("p (t b) -> p b t", b=BL)
        nc.vector.tensor_reduce(mpred[:, :], oh_b[:, :, :],
                                mybir.AxisListType.X, OP.add)
        mw_b = mw.rearrange("p (t b) -> p b t", b=BL)
        xt6_b = xt.rearrange("p k (t b) -> p k b t", b=BL)
        for b in range(BL):
            nc.vector.tensor_scalar(xt6_b[0:1, 6, b, :], mw_b[:, b, :],
                                    mpred[0:1, b:b + 1], None, OP.subtract)

        # ---- projections + scan -----------------------------------------
        def projection(wih_dram, nk, rhs_of_k, xw, chunks=(0, 1, 2, 3)):
            """xw[:, m, :] (bf16 [128, MT, SB]) = Wih_ext.T @ rhs."""
            for m in range(MT):
                wm = wpool.tile([128, nk, 128], BF16, tag="wihm")
                nc.sync.dma_start(out=wm[:, :, :], in_=wih_dram[m])
                for ch in chunks:
                    pp = pspool.tile([128, 512], F32, tag="proj")
                    for k in range(nk):
                        nc.tensor.matmul(pp[:, :], wm[:, k, :], rhs_of_k(k, ch),
                                         start=(k == 0), stop=(k == nk - 1))
                    nc.vector.tensor_copy(xw[:, m, 512 * ch:512 * (ch + 1)],
                                          pp[:, :])

        def make_filler(wih_dram, nk, rhs_of_k, xw, chunks, quota=4):
            """Per-step emitter of projection matmuls into the scan's PE
            tail. Chunk-major so chunk c completes well before the scan
            reaches its columns. Weight tiles are prefetched one task
            ahead; the psum->xw cast runs on gpsimd to keep the DVE chain
            clean."""
            tasks = [(ch, m) for ch in chunks for m in range(MT)]
            stt = {"ti": 0, "k": 0, "pp": None, "wms": {}}

            def ensure_wm(ti):
                if ti < len(tasks) and ti not in stt["wms"]:
                    t = wpool.tile([128, nk, 128], BF16, tag="wihm",
                                   name="fillwm")
                    nc.sync.dma_start(out=t[:, :, :], in_=wih_dram[tasks[ti][1]])
                    stt["wms"][ti] = t

            def fill(j):
                budget = quota
                while budget > 0 and stt["ti"] < len(tasks):
                    ti = stt["ti"]
                    ch, m = tasks[ti]
                    ensure_wm(ti)
                    ensure_wm(ti + 1)
                    wm = stt["wms"][ti]
                    if stt["k"] == 0:
                        stt["pp"] = pspool.tile([128, 512], F32, tag="proj",
                                                name="fillpp")
                    take = min(budget, nk - stt["k"])
                    for k in range(stt["k"], stt["k"] + take):
                        nc.tensor.matmul(stt["pp"][:, :], wm[:, k, :],
                                         rhs_of_k(k, ch),
                                         start=(k == 0), stop=(k == nk - 1))
                    stt["k"] += take
                    budget -= take
                    if stt["k"] == nk:
                        nc.vector.tensor_copy(
                            xw[:, m, 512 * ch:512 * (ch + 1)], stt["pp"][:, :])
                        del stt["wms"][ti]
                        stt["ti"] += 1
                        stt["k"] = 0

            return fill

        def scan_layer(whh_sb, xw, hdst, fill=None):
            """Single-direction 256-step scan. Gate groups g,i,f emitted
            k-outer (their psums complete at fixed points of the stream);
            o-gate m-outer and post-processed in two m-slices so h's k-tiles
            release progressively; next step's k-outer matmuls then need the
            late k-tiles last. The hdst copy is deferred one step (on
            gpsimd) so it runs under the following MM stream."""
            hbuf = rpool.tile([128, 2, KH, BL], BF16, tag="hbuf", name="hbuf")
            nc.vector.memset(hbuf[:, 0, :, :], 0.0)
            cbuf = rpool.tile([128, KH, BL], F32, tag="cbuf", name="cbuf")
            nc.vector.memset(cbuf[:, :, :], 0.0)

            prev = None
            for j in range(S):
                cur, nxt = j % 2, (j + 1) % 2
                col = j * BL
                # stream order: i,f first, then g, then o. Each group's
                # xw term is accumulated INTO its psum by one identity
                # matmul that also closes the accumulation, so the
                # activations read PSUM directly (no DVE adds).
                pgif = psg.tile([128, 8, BL], F32, tag="gif", name="pgif")
                for mm in range(8):            # i, f = m-tiles 0..7
                    for k in range(KH):
                        nc.tensor.matmul(
                            pgif[:, mm, :],
                            whh_sb[:, k, 128 * mm:128 * (mm + 1)],
                            hbuf[:, cur, k, :],
                            start=(k == 0 and mm == 0), stop=False)
                nc.tensor.matmul(pgif[:, :, :], identb[:, :],
                                 xw[:, 0:8, ds(col, BL)],
                                 start=False, stop=True)
                pgg = psg.tile([128, KH, BL], F32, tag="gg", name="pgg")
                for mm in range(KH):           # g = m-tiles 8..11
                    m = 8 + mm
                    for k in range(KH):
                        nc.tensor.matmul(
                            pgg[:, mm, :],
                            whh_sb[:, k, 128 * m:128 * (m + 1)],
                            hbuf[:, cur, k, :],
                            start=(k == 0 and mm == 0), stop=False)
                nc.tensor.matmul(pgg[:, :, :], identb[:, :],
                                 xw[:, 8:12, ds(col, BL)],
                                 start=False, stop=True)
                pgo = psg.tile([128, KH, BL], F32, tag="go", name="pgo")
                for mm in range(KH):           # o = m-tiles 12..15
                    m = 12 + mm
                    for k in range(KH):
                        nc.tensor.matmul(
                            pgo[:, mm, :],
                            whh_sb[:, k, 128 * m:128 * (m + 1)],
                            hbuf[:, cur, k, :],
                            start=(k == 0 and mm == 0), stop=False)
                nc.tensor.matmul(pgo[:, :, :], identb[:, :],
                                 xw[:, 12:16, ds(col, BL)],
                                 start=False, stop=True)
                if prev is not None:
                    nc.gpsimd.tensor_copy(hdst[:, :, ds(prev[0] * BL, BL)],
                                          hbuf[:, prev[1], :, :])
                if fill is not None:
                    fill(j)
                sif = scpool.tile([128, 8, BL], F32, tag="sif")
                nc.scalar.activation(sif[:, :, :], pgif[:, :, :],
                                     AF.Sigmoid)
                tg = scpool.tile([128, KH, BL], F32, tag="tg", name="tg")
                nc.scalar.activation(tg[:, :, :], pgg[:, :, :], AF.Tanh)
                so = scpool.tile([128, KH, BL], F32, tag="so")
                nc.scalar.activation(so[:, :, :], pgo[:, :, :], AF.Sigmoid)
                t2 = scpool.tile([128, KH, BL], F32, tag="t2")
                nc.vector.tensor_tensor(t2[:, :, :], sif[:, 4:8, :],
                                        cbuf[:, :, :], OP.mult)
                t1 = scpool.tile([128, KH, BL], F32, tag="t1", name="t1")
                nc.vector.tensor_tensor(t1[:, :, :], sif[:, 0:4, :],
                                        tg[:, :, :], OP.mult)
                nc.vector.tensor_tensor(cbuf[:, :, :], t1[:, :, :],
                                        t2[:, :, :], OP.add)
                tcc = scpool.tile([128, KH, BL], F32, tag="tcc", name="tcc")
                nc.scalar.activation(tcc[:, :, :], cbuf[:, :, :], AF.Tanh)
                nc.vector.tensor_tensor(hbuf[:, nxt, :, :], so[:, :, :],
                                        tcc[:, :, :], OP.mult)
                prev = (j, nxt)
            nc.gpsimd.tensor_copy(hdst[:, :, ds(prev[0] * BL, BL)],
                                  hbuf[:, prev[1], :, :])

        def exchange_half(hown, hrecv, tag, lo, hi):
            """Pair AllGather of hown cols [lo:hi); select partner slot and
            time-reverse into hrecv cols [SB-hi:SB-lo). Issued high-half
            first so downstream consumers of hrecv's low columns can start
            while the second half is still in flight."""
            W = hi - lo
            hexp = dpool.tile([128, KH, W], BF16, tag=f"hx{lo}")
            nc.sync.dma_start(out=hexp[:, :, :], in_=hown[:, :, lo:hi])
            hgat = dpool.tile([2, 128, KH, W], BF16, tag=f"hg{lo}")
            nc.gpsimd.collective_compute(
                "AllGather", mybir.AluOpType.bypass,
                replica_groups=GROUPS,
                ins=[hexp.opt()],
                outs=[hgat.opt()],
            )
            r = []
            for s in range(2):
                rt = xpool.tile([128, KH, W], BF16, tag=f"xr{s}_{lo}",
                                name=f"{tag}r{s}_{lo}")
                nc.sync.dma_start(out=rt[:, :, :], in_=hgat[s])
                r.append(rt)
            r0v = r[0].rearrange("p k (t b) -> p k t b", b=BL)
            r1v = r[1].rearrange("p k (t b) -> p k t b", b=BL)
            hv = hrecv.rearrange("p k (t b) -> p k t b", b=BL)
            tsl = slice((SB - hi) // BL, (SB - lo) // BL)
            nc.vector.tensor_scalar(hv[:, :, tsl, :], r0v[:, :, ::-1, :],
                                    mscal[:, 0:1], None, OP.mult)
            nc.vector.tensor_scalar(r[0][:, :, :], r1v[:, :, ::-1, :],
                                    mscal[:, 1:2], None, OP.mult)
            nc.vector.tensor_tensor(hrecv[:, :, SB - hi:SB - lo],
                                    hrecv[:, :, SB - hi:SB - lo],
                                    r[0][:, :, :], OP.add)

        # ---- layer 0 -----------------------------------------------------
        whh0 = whpool.tile([128, KH, G], BF16, tag="whh", name="whh0")
        nc.sync.dma_start(out=whh0[:, :, :], in_=w["whh0"][:, :, :])
        xw0 = xwpool.tile([128, MT, SB], BF16, tag="xw", name="xw0")
        rhs0 = lambda k, ch: xt[:, k, 512 * ch:512 * (ch + 1)]
        projection(w["wih0"], K0, rhs0, xw0, chunks=(0,))
        h0own = rpool.tile([128, KH, SB], BF16, tag="hA", name="h0own")
        scan_layer(whh0, xw0, h0own,
                   fill=make_filler(w["wih0"], K0, rhs0, xw0, (1, 2, 3)))
        fst.close()

        # ---- exchange h0 (two halves, high first) ------------------------
        xpool = st.enter_context(tc.tile_pool(name="xch", bufs=1))
        h0recv = rpool.tile([128, KH, SB], BF16, tag="hB", name="h0recv")
        exchange_half(h0own, h0recv, "h0", SB // 2, SB)
        exchange_half(h0own, h0recv, "h0", 0, SB // 2)

        # ---- layer 1 -----------------------------------------------------
        whh1 = whpool.tile([128, KH, G], BF16, tag="whh", name="whh1")
        nc.sync.dma_start(out=whh1[:, :, :], in_=w["whh1"][:, :, :])

        def l1_rhs(k, ch):
            if k < KH:
                return h0own[:, k, 512 * ch:512 * (ch + 1)]
            if k < 2 * KH:
                return h0recv[:, k - KH, 512 * ch:512 * (ch + 1)]
            return ones_row[:, 0:512]

        xw1 = xwpool.tile([128, MT, SB], BF16, tag="xw", name="xw1")
        projection(w["wih1"], K1, l1_rhs, xw1, chunks=(0,))
        h1own = rpool.tile([128, KH, SB], BF16, tag="hA", name="h1own")
        scan_layer(whh1, xw1, h1own,
                   fill=make_filler(w["wih1"], K1, l1_rhs, xw1, (1, 2, 3)))

        # ---- output projection via partial exchange ----------------------
        wo = wpool.tile([128, 5, C], BF16, tag="wout")
        nc.sync.dma_start(out=wo[:, :, :], in_=w["wout"][:, :, :])
        outP = rpool.tile([C, SB], F32, tag="outP")
        for ch in range(4):
            po = pspool.tile([C, 512], F32, tag="proj")
            for k in range(5):
                rhs = (h1own[:, k, 512 * ch:512 * (ch + 1)] if k < KH
                       else ones_row[:, 0:512])
                nc.tensor.matmul(po[:, :], wo[:, k, :], rhs,
                                 start=(k == 0), stop=(k == 4))
            nc.vector.tensor_copy(outP[:, 512 * ch:512 * (ch + 1)], po[:, :])
        pexp = dpool.tile([C, SB], F32, tag="pexp")
        nc.sync.dma_start(out=pexp[:, :], in_=outP[:, :])
        pgat = dpool.tile([2, C, SB], F32, tag="pgat")
        nc.gpsimd.collective_compute(
            "AllGather", mybir.AluOpType.bypass,
            replica_groups=GROUPS,
            ins=[pexp.opt()],
            outs=[pgat.opt()],
        )
        pr = []
        for s in range(2):
            rt = xpool.tile([C, SB], F32, tag=f"pr{s}", name=f"pr{s}")
            nc.sync.dma_start(out=rt[:, :], in_=pgat[s])
            pr.append(rt)
        outT = rpool.tile([C, SB], F32, tag="outT")
        r0v = pr[0].rearrange("p (t b) -> p t b", b=BL)
        r1v = pr[1].rearrange("p (t b) -> p t b", b=BL)
        oTv = outT.rearrange("p (t b) -> p t b", b=BL)
        nc.vector.tensor_scalar(oTv[:, :, :], r0v[:, ::-1, :],
                                mscal[0:C, 0:1], None, OP.mult)
        nc.vector.tensor_scalar(pr[0][:, :], r1v[:, ::-1, :],
                                mscal[0:C, 1:2], None, OP.mult)
        nc.vector.tensor_tensor(outT[:, :], outT[:, :], pr[0][:, :], OP.add)
        nc.vector.tensor_tensor(outT[:, :], outT[:, :], outP[:, :], OP.add)
        out_sbc = out.rearrange("b s c -> s b c")
        for cb in range(SB // 128):
            pt = pspool.tile([128, C], F32, tag="tp")
            nc.tensor.transpose(pt[:, :], outT[:, 128 * cb:128 * (cb + 1)],
                                ident[0:C, 0:C])
            onat = scpool.tile([128, C], F32, tag="onat")
            nc.vector.tensor_copy(onat[:, :], pt[:, :])
            nc.sync.dma_start(out=out_sbc[SR * cb:SR * (cb + 1), :, :],
                              in_=onat[:, :])


def _prep_core_inputs(inputs, wmaps=None):
    """Host-side: per-core input maps (8 cores)."""
    if wmaps is None:
        wmaps = _prep_weights(inputs)
    hsf = np.asarray(inputs["hidden_states"], np.float32)
    rol = np.asarray(inputs["roles"])
    prd = np.asarray(inputs["predicates"])
    maskf = ((rol != 0) & (rol != -100)).astype(np.float32)    # [B,S]
    idx = np.argmax(prd, axis=-1)                              # [B]
    ohf = np.zeros((B, S), np.float32)
    ohf[np.arange(B), idx] = 1.0
    in_maps = []
    for c in range(NCORES):
        g, d = c // 2, c % 2
        sl = slice(BL * g, BL * (g + 1))
        m = dict(wmaps["f" if d == 0 else "b"])
        hsc = hsf[:, sl]
        mk = maskf[sl]
        ohc = ohf[sl]
        if d == 1:                                   # time-reverse for bwd
            hsc = hsc[:, :, ::-1]
            mk = mk[:, ::-1]
            ohc = ohc[:, ::-1]
        m["hs"] = _bf(np.ascontiguousarray(hsc))                # [4,BL,S,E]
        m["mask"] = _bf(np.ascontiguousarray(mk.T.reshape(1, SB)))  # (t,b)
        m["oh"] = np.ascontiguousarray(ohc.T.reshape(1, SB))
        mrow_ = (np.array([[0.0, 1.0]], np.float32) if d == 0
                 else np.array([[1.0, 0.0]], np.float32))
        m["msel"] = np.ascontiguousarray(np.repeat(mrow_, 128, axis=0))
        in_maps.append(m)
    return in_maps


def _get_nc():
    if "nc" not in _cache:
        _cache["nc"] = build_nc()
    return _cache["nc"]


def kernel(**inputs):
    from concourse.bass_utils import run_bass_kernel_spmd

    in_maps = _prep_core_inputs(inputs)
    nc = _get_nc()
    res = run_bass_kernel_spmd(nc, in_maps, core_ids=list(range(NCORES)))
    # even cores carry the true-time output for their batch group
    return np.concatenate([res.results[2 * g]["out"] for g in range(NPAIR)],
                          axis=0)
